# revision 35
# baseline (speedup 1.0000x reference)
"""Trainium2 Bass kernel for nn_HausdorffDistance_28406913696124.

Math (reference):
    px = (prob_map[0].ravel() >= 0.5)                 # [N], N = 100*100
    py = (gt_map.ravel()   >= 0.5)                    # [N]
    D[i,j] = euclid dist between grid points i, j     # [N, N] constant!
    loss   = mean_i | px_i * mean_j D[i,j] - (D @ py)_i / N |

Key structure:
  * rowmean_i = mean_j D[i,j] is a pure geometric constant -> host table.
  * (D @ py) is a 2D correlation of the binary mask PY with the 199x199
    kernel K(u,v) = sqrt(u^2 + v^2).  K is numerically low-rank: with a
    displacement-count weighted SVD, rank 2 already reproduces the final
    scalar to ~1e-5 relative (tolerance is 2e-2).  K ~= sum_m f_m(u) g_m(v)
    turns the 2D correlation into two passes of 1D Toeplitz matmuls:

        G_m = PY^T-contraction:  G_m[a, c] = sum_b PYT[b, a] * g_m(c - b)
        T2[r, c] = sum_m sum_a f_m(r - a) * G_m[a, c]

    Stage 1 is ONE matmul (stationary = binarized transposed mask, moving =
    the [100, 100*M] constant Toeplitz table); stage 2 is M accumulating
    matmuls with constant stationaries.  The gt transpose is done on host so
    no on-device transposes are needed.

  * All scalar factors (the two 1/N means) are folded into the constant
    tables (1e-4 into each of the stage-1/stage-2 tables, -1e-8 into the
    row-sum table), so after |.| row-reduction a single ones-matmul across
    partitions yields the final loss directly -- no post-scale.

  * term1 is folded in by pre-writing t1n = px * (-1e-8 * rowsum) into the
    stage-2 PSUM accumulation bank before the matmuls (start=False), so the
    bank ends up holding diff = 1e-8*(T2sum - px*rowsumN) elementwise.

Single core, no collectives: the whole problem is ~0.5 MFLOP, so the 8-core
AllReduce floor (~5us) dwarfs any compute sharding gain.  The kernel is
latency-bound: two parallel input DMAs (SP ring + Pool SWDGE ring), ~0.6us
of matmul/DVE work, one 4-byte output DMA.
"""

import sys

import numpy as np

sys.path.insert(0, "/opt/trn_rl_repo")

H = 100
N = H * H
RANK = 1      # separable ranks of the distance kernel (1 -> ~4e-4 rel err,
              # 2 -> ~2e-5; tolerance is 2e-2)
S_A = 1e-4    # scale folded into the stage-1 moving table
S_B = 1e-4    # scale folded into the stage-2 stationary table
PAD2 = 8      # f16 cols holding the f32 ones column (4) + pad


def _host_constants():
    """Geometry-only constant tables (input independent)."""
    idx = np.arange(H)

    # Displacement-count weighted SVD of K(u,v) = sqrt(u^2+v^2) on
    # [-99,99]^2: weight sqrt(100-|u|) per axis makes the truncation error
    # small exactly where displacements are frequent.
    u = np.arange(-(H - 1), H)
    K = np.sqrt((u[:, None] ** 2 + u[None, :] ** 2).astype(np.float64))
    wu = np.sqrt((H - np.abs(u)).astype(np.float64))
    Uw, S, Vtw = np.linalg.svd(wu[:, None] * K * wu[None, :])
    ffac = (Uw * np.sqrt(S)[None, :]) / wu[:, None]    # [199, m]
    gfac = (Vtw.T * np.sqrt(S)[None, :]) / wu[:, None]  # [199, m]

    # Toeplitz tables: offs[a, r] = (r - a) + 99
    offs = (idx[None, :] - idx[:, None]) + (H - 1)
    m1 = np.concatenate(
        [gfac[:, k][offs] * S_A for k in range(RANK)], axis=1
    ).astype(np.float16)                               # [100, 100*RANK]
    ftab = np.concatenate(
        [ffac[:, k][offs] * S_B for k in range(RANK)], axis=1
    ).astype(np.float16)                               # [100, 100*RANK]

    # rowsum[r,c] = sum_j D[i,j] (i = r*100+c), f64-exact, scaled by -1e-8
    # so the DVE write into the accumulation bank is already -px*rowsum/N^2.
    absdiff = np.abs(idx[:, None] - idx[None, :])
    q = np.sqrt((idx[:, None] ** 2 + idx[None, :] ** 2).astype(np.float64))
    cnt = np.zeros((H, H))
    np.add.at(cnt, (idx[:, None], absdiff), 1.0)
    rm_tbl = (-(cnt @ q @ cnt.T) * (S_A * S_B)).astype(np.float32)

    # pack2 = [ftab | rm(f32) | prob placeholder(f32)]
    const2 = np.concatenate(
        [
            ftab,
            rm_tbl.view(np.float16),
            np.zeros((H, 2 * H), dtype=np.float16),  # prob slot
        ],
        axis=1,
    )
    return m1, const2


def _build_module():
    import concourse.bacc as bacc
    import concourse.bass as bass_mod
    import concourse.mybir as mybir
    import concourse.tile as tile

    f32 = mybir.dt.float32
    f16 = mybir.dt.float16

    # Bass.__init__ emits four const-AP memsets, all on Pool; Pool then
    # joins the prologue all-engine barrier ~400ns after everyone else,
    # delaying the whole program.  Spread the two f32 consts across DVE
    # and Pool, and skip the bf16-1.0 / uint8-127 fills entirely -- no
    # instruction in this program touches those dtypes, so the (still
    # allocated) regions are never read.
    _orig_memset = bass_mod.BassGpSimd.memset
    _ctr = [0]

    def _memset_rr(self, ap, constant):
        _ctr[0] += 1
        if _ctr[0] % 2:
            return bass_mod.BassVectorEngine.memset(self.bass.vector, ap, constant)
        return _orig_memset(self, ap, constant)

    bass_mod.BassGpSimd.memset = _memset_rr
    try:
        nc = bacc.Bacc(
            "TRN2",
            target_bir_lowering=False,
            debug=False,
            enable_asserts=False,
            num_devices=1,
        )
    finally:
        bass_mod.BassGpSimd.memset = _orig_memset

    W1 = 2 * H + RANK * H          # pack1: gtT (f32 as 2H f16) | M1
    W2 = RANK * H + 4 * H          # pack2: ftab | rm | prob
    pack1_d = nc.dram_tensor("pack1", [H, W1], f16, kind="ExternalInput")
    pack2_d = nc.dram_tensor("pack2", [H, W2], f16, kind="ExternalInput")
    # kv_writeback stores a full 128-partition column; slot 0 is the answer.
    out_d = nc.dram_tensor("out", [1, 128], f32, kind="ExternalOutput")

    dma_sem = nc.alloc_semaphore("out_wb_done")
    out_val = nc.alloc_sbuf_tensor("out_val", [128, 1], f32)

    with tile.TileContext(nc) as tc:
        with (
            tc.tile_pool(name="sb", bufs=1) as sb,
            tc.tile_pool(name="ps_g", bufs=1, space="PSUM") as ps_g,
            tc.tile_pool(name="ps_acc", bufs=1, space="PSUM") as ps_acc,
        ):
            # ---- loads: critical (gtT|M1) on the SP HWDGE ring, the rest
            # on the Pool SWDGE ring so neither waits on the other --------
            pk1 = sb.tile([H, W1], f16)
            nc.sync.dma_start(pk1[:], pack1_d[:])
            pk2 = sb.tile([H, W2], f16)
            nc.gpsimd.dma_start(pk2[:], pack2_d[:])

            # ---- output writeback, prepared early: the SWDGE descriptors
            # are generated during Pool's idle window, so after the final
            # reduce only the trigger (SEQ op + transfer + completion sem)
            # is left -- skipping the ~1.3us HWDGE+DGE serial pipeline.
            # out_val is a RAW sbuf tensor (not a tile): Tile must not see
            # the prep's deferred read, or it inserts a WAR wait on the
            # transfer completion in front of the reduce (deadlock).  The
            # read-after-write ordering is enforced at the trigger instead
            # (signals_writable on the PSUM bank the reduce consumes). ----
            out_ap = out_val.ap()
            wb_idx = sb.tile([128, 1], mybir.dt.int32)
            nc.vector.memset(out_ap, 0.0)
            nc.vector.memset(wb_idx[:], 0)
            nc.gpsimd.kv_writeback(
                out_d[:].rearrange("b (d o n) -> b d o n", o=1, n=1),
                out_ap.rearrange("d (o b n) -> d o b n", b=1, n=1),
                wb_idx[:],
                prepare_only=True,
                sem=dma_sem,
            )

            gtT = pk1[:, 0:2 * H].bitcast(f32)            # [100,100] f32
            m1 = pk1[:, 2 * H:2 * H + RANK * H]           # [100,100R] f16
            ftab = pk2[:, 0:RANK * H]                     # [100,100R] f16
            rm = pk2[:, RANK * H:RANK * H + 2 * H].bitcast(f32)
            prob = pk2[:, RANK * H + 2 * H:RANK * H + 4 * H].bitcast(f32)

            # ---- binarize the transposed mask (stage-1 stationary) ------
            pyt = sb.tile([H, H], f16)
            nc.vector.tensor_scalar(
                pyt[:], gtT, 0.5, None, mybir.AluOpType.is_ge
            )

            # ---- stage 1: G[a, mc] = sum_b pyt[b,a] * m1[b, mc] ---------
            g_ps = ps_g.tile([H, RANK * H], f32)
            nc.tensor.matmul(g_ps[:], pyt[:], m1, start=True, stop=True)

            # G-copy on ACT (otherwise idle; scalar_tensor_tensor is not
            # legal on Pool, so t1n needs DVE and the copy moves off it).
            g16 = sb.tile([H, RANK * H], f16)
            nc.scalar.copy(g16[:], g_ps[:])

            # ---- t1n = px * (-rowsum/N^2) pre-written into the stage-2
            # accumulation bank (matmuls then add on top, start=False) ----
            acc_ps = ps_acc.tile([H, H], f32)
            nc.vector.scalar_tensor_tensor(
                acc_ps[:],
                prob,
                0.5,
                rm,
                op0=mybir.AluOpType.is_ge,
                op1=mybir.AluOpType.mult,
            )

            # ---- stage 2: acc += sum_m ftab_m^T @ G_m -------------------
            for k in range(RANK):
                nc.tensor.matmul(
                    acc_ps[:],
                    ftab[:, k * H:(k + 1) * H],
                    g16[:, k * H:(k + 1) * H],
                    start=False,
                    stop=(k == RANK - 1),
                    skip_group_check=True,
                )

            # ---- final: sum_i |acc_i| (scales already folded).  Pool's
            # partition-axis reduce cannot read PSUM, so: DVE abs-reduce
            # along the free axis (PSUM -> [100,1] SBUF), then a tiny Pool
            # partition reduce to the scalar -- no ones-matmul. -----------
            rowsums = sb.tile([H, 1], f32)
            nc.vector.tensor_reduce(
                rowsums[:],
                acc_ps[:],
                axis=mybir.AxisListType.X,
                op=mybir.AluOpType.add,
                apply_absolute_value=True,
            )
            nc.gpsimd.tensor_reduce(
                out_ap[0:1, 0:1],
                rowsums[:],
                axis=mybir.AxisListType.XYZWC,
                op=mybir.AluOpType.add,
            )
            # Fire the pre-built writeback.  signals_writable=[rowsums]
            # gives the trigger a WAR dep on the Pool reduce's read so it
            # cannot fire before the scalar is written.  The epilogue's
            # SWDGE-queue drain (DMASW lane sem, rewritten below) holds
            # program end until the transfer completes.
            nc.gpsimd.trigger_dma(signals_writable=[rowsums[:]])

    nc.compile()

    # Post-compile surgery (compile regenerates event-semaphore waits, so
    # this must run on the final BIR).
    #
    # Tile schedules the kv-writeback prep on a dedicated DMASW lane and
    # makes the epilogue wait for that lane's completion sem (+16), but the
    # descriptor-baked completion sem is the one passed as `sem=` above.
    # Rewrite the prep's completion update to target the (otherwise
    # orphaned) DMASW lane sem so the epilogue wait is satisfied by the
    # actual transfer completion.
    updated_ids = set()
    waited = {}
    prep = None
    for bb in nc.m.functions[0].blocks:
        for inst in bb.instructions:
            if type(inst).__name__ == "InstKVWritebackAnt":
                prep = inst
            si = inst.sync_info
            if si is None:
                continue
            for u in si.on_update:
                updated_ids.add(u.id)
            for w in si.on_wait:
                if w.ant_name and w.ant_name.startswith("DMASW"):
                    waited[w.id] = w.ant_name
    orphans = {i: n for i, n in waited.items() if i not in updated_ids}
    assert prep is not None and len(orphans) == 1, (prep, orphans)
    ((oid, oname),) = orphans.items()
    si = prep.sync_info
    old = list(si.on_update)
    si.on_update = [
        mybir.SyncUpdate(
            sync_type="semaphore",
            id=oid,
            ant_name=oname,
            update_mode="sem-add-imm",
            update_value=16,
            update_reg=None,
        )
    ] + old[1:]

    # Tile also guards the Pool Q7 library reload (for the XYZWC reduce)
    # behind "SWDGE queue drained" -- circular with a prepared-untriggered
    # DMA.  The guard is unnecessary for this pattern (the descriptors
    # live in the SBUF ring, not Q7 memory): strip the orphan-lane wait
    # from body-block event semaphores.  The epilogue's drain waits (which
    # hold program end until the transfer lands) are left untouched.
    for bb in nc.m.functions[0].blocks:
        if not (bb.name.startswith("tile_context") and not bb.name.endswith("_end")):
            continue
        for inst in bb.instructions:
            if type(inst).__name__ != "InstEventSemaphore":
                continue
            si2 = inst.sync_info
            if si2 is None:
                continue
            kept = [w for w in si2.on_wait if w.id != oid]
            if len(kept) != len(list(si2.on_wait)):
                si2.on_wait = kept

    # The epilogue runs TWO all-engine barrier rounds: one before the Pool
    # sem-cleanup ISA (needed -- it guarantees quiescence), and a second
    # after it whose only purpose is to delay every engine's halt until
    # the cleanup ran.  Engines halting at different times is fine (the
    # program is over; cleanup still completes before Pool halts), so
    # neuter the second round's synchronization.
    for bb in nc.m.functions[0].blocks:
        if not bb.name.endswith("_end"):
            continue
        seen_isa = False
        for inst in bb.instructions:
            tn = type(inst).__name__
            if tn == "InstISA":
                seen_isa = True
                continue
            if seen_isa and tn in ("InstDrain", "InstEventSemaphore"):
                si3 = inst.sync_info
                if si3 is not None:
                    si3.on_wait = []
                    si3.on_update = []

    return nc


_STATE = {}


def _get_state():
    if not _STATE:
        _STATE["consts"] = _host_constants()
        _STATE["nc"] = _build_module()
    return _STATE


def _in_maps(prob_map, gt_map):
    st = _get_state()
    m1, const2 = st["consts"]
    prob = np.asarray(prob_map, dtype=np.float32).reshape(H, H)
    gt = np.asarray(gt_map, dtype=np.float32).reshape(H, H)
    gtT = np.ascontiguousarray(gt.T)

    pack1 = np.concatenate([gtT.view(np.float16), m1], axis=1)
    pack2 = const2.copy()
    pack2[:, RANK * H + 2 * H:RANK * H + 4 * H] = prob.view(np.float16)
    return [{"pack1": np.ascontiguousarray(pack1),
             "pack2": np.ascontiguousarray(pack2)}]


def _run(prob_map, gt_map, trace=False, **spmd_kwargs):
    from concourse import bass_utils

    st = _get_state()
    in_maps = _in_maps(prob_map, gt_map)
    res = bass_utils.run_bass_kernel_spmd(
        st["nc"], in_maps, core_ids=[0], trace=trace, **spmd_kwargs,
    )
    value = np.float32(np.asarray(res.results[0]["out"]).ravel()[0])
    return value, res


def kernel(prob_map, gt_map):
    value, _ = _run(prob_map, gt_map, trace=False)
    return np.asarray(value, dtype=np.float32)


# revision 37
# speedup vs baseline: 1.0408x; 1.0408x over previous
"""Trainium2 Bass kernel for nn_HausdorffDistance_28406913696124.

Math (reference):
    px = (prob_map[0].ravel() >= 0.5)                 # [N], N = 100*100
    py = (gt_map.ravel()   >= 0.5)                    # [N]
    D[i,j] = euclid dist between grid points i, j     # [N, N] constant!
    loss   = mean_i | px_i * mean_j D[i,j] - (D @ py)_i / N |

Key structure:
  * rowmean_i = mean_j D[i,j] is a pure geometric constant -> host table.
  * (D @ py) is a 2D correlation of the binary mask PY with the 199x199
    kernel K(u,v) = sqrt(u^2 + v^2).  K is numerically low-rank: with a
    displacement-count weighted SVD, rank 2 already reproduces the final
    scalar to ~1e-5 relative (tolerance is 2e-2).  K ~= sum_m f_m(u) g_m(v)
    turns the 2D correlation into two passes of 1D Toeplitz matmuls:

        G_m = PY^T-contraction:  G_m[a, c] = sum_b PYT[b, a] * g_m(c - b)
        T2[r, c] = sum_m sum_a f_m(r - a) * G_m[a, c]

    Stage 1 is ONE matmul (stationary = binarized transposed mask, moving =
    the [100, 100*M] constant Toeplitz table); stage 2 is M accumulating
    matmuls with constant stationaries.  The gt transpose is done on host so
    no on-device transposes are needed.

  * All scalar factors (the two 1/N means) are folded into the constant
    tables (1e-4 into each of the stage-1/stage-2 tables, -1e-8 into the
    row-sum table), so after |.| row-reduction a single ones-matmul across
    partitions yields the final loss directly -- no post-scale.

  * term1 is folded in by pre-writing t1n = px * (-1e-8 * rowsum) into the
    stage-2 PSUM accumulation bank before the matmuls (start=False), so the
    bank ends up holding diff = 1e-8*(T2sum - px*rowsumN) elementwise.

Single core, no collectives: the whole problem is ~0.5 MFLOP, so the 8-core
AllReduce floor (~5us) dwarfs any compute sharding gain.  The kernel is
latency-bound: two parallel input DMAs (SP ring + Pool SWDGE ring), ~0.6us
of matmul/DVE work, one 4-byte output DMA.
"""

import sys

import numpy as np

sys.path.insert(0, "/opt/trn_rl_repo")

H = 100
N = H * H
RANK = 1      # separable ranks of the distance kernel (1 -> ~4e-4 rel err,
              # 2 -> ~2e-5; tolerance is 2e-2)
S_A = 1e-4    # scale folded into the stage-1 moving table
S_B = 1e-4    # scale folded into the stage-2 stationary table
PAD2 = 8      # f16 cols holding the f32 ones column (4) + pad


def _host_constants():
    """Geometry-only constant tables (input independent)."""
    idx = np.arange(H)

    # Displacement-count weighted SVD of K(u,v) = sqrt(u^2+v^2) on
    # [-99,99]^2: weight sqrt(100-|u|) per axis makes the truncation error
    # small exactly where displacements are frequent.
    u = np.arange(-(H - 1), H)
    K = np.sqrt((u[:, None] ** 2 + u[None, :] ** 2).astype(np.float64))
    wu = np.sqrt((H - np.abs(u)).astype(np.float64))
    Uw, S, Vtw = np.linalg.svd(wu[:, None] * K * wu[None, :])
    ffac = (Uw * np.sqrt(S)[None, :]) / wu[:, None]    # [199, m]
    gfac = (Vtw.T * np.sqrt(S)[None, :]) / wu[:, None]  # [199, m]

    # Toeplitz tables: offs[a, r] = (r - a) + 99
    offs = (idx[None, :] - idx[:, None]) + (H - 1)
    m1 = np.concatenate(
        [gfac[:, k][offs] * S_A for k in range(RANK)], axis=1
    ).astype(np.float16)                               # [100, 100*RANK]
    ftab = np.concatenate(
        [ffac[:, k][offs] * S_B for k in range(RANK)], axis=1
    ).astype(np.float16)                               # [100, 100*RANK]

    # rowsum[r,c] = sum_j D[i,j] (i = r*100+c), f64-exact, scaled by -1e-8
    # so the DVE write into the accumulation bank is already -px*rowsum/N^2.
    absdiff = np.abs(idx[:, None] - idx[None, :])
    q = np.sqrt((idx[:, None] ** 2 + idx[None, :] ** 2).astype(np.float64))
    cnt = np.zeros((H, H))
    np.add.at(cnt, (idx[:, None], absdiff), 1.0)
    rm_tbl = (-(cnt @ q @ cnt.T) * (S_A * S_B)).astype(np.float32)

    # pack2 = [ftab | rm(f32) | prob placeholder(f32)]
    const2 = np.concatenate(
        [
            ftab,
            rm_tbl.view(np.float16),
            np.zeros((H, 2 * H), dtype=np.float16),  # prob slot
        ],
        axis=1,
    )
    return m1, const2


def _build_module():
    import concourse.bacc as bacc
    import concourse.bass as bass_mod
    import concourse.mybir as mybir
    import concourse.tile as tile

    f32 = mybir.dt.float32
    f16 = mybir.dt.float16

    # Bass.__init__ emits four const-AP memsets, all on Pool; Pool then
    # joins the prologue all-engine barrier ~400ns after everyone else,
    # delaying the whole program.  Spread the two f32 consts across DVE
    # and Pool, and skip the bf16-1.0 / uint8-127 fills entirely -- no
    # instruction in this program touches those dtypes, so the (still
    # allocated) regions are never read.
    _orig_memset = bass_mod.BassGpSimd.memset
    _ctr = [0]

    def _memset_rr(self, ap, constant):
        _ctr[0] += 1
        if _ctr[0] % 2:
            return bass_mod.BassVectorEngine.memset(self.bass.vector, ap, constant)
        return _orig_memset(self, ap, constant)

    bass_mod.BassGpSimd.memset = _memset_rr
    try:
        nc = bacc.Bacc(
            "TRN2",
            target_bir_lowering=False,
            debug=False,
            enable_asserts=False,
            num_devices=1,
        )
    finally:
        bass_mod.BassGpSimd.memset = _orig_memset

    W1 = 2 * H + RANK * H          # pack1: gtT (f32 as 2H f16) | M1
    W2 = RANK * H + 4 * H          # pack2: ftab | rm | prob
    pack1_d = nc.dram_tensor("pack1", [H, W1], f16, kind="ExternalInput")
    pack2_d = nc.dram_tensor("pack2", [H, W2], f16, kind="ExternalInput")
    # kv_writeback stores a full 128-partition column; slot 0 is the answer.
    out_d = nc.dram_tensor("out", [1, 128], f32, kind="ExternalOutput")

    dma_sem = nc.alloc_semaphore("out_wb_done")
    out_val = nc.alloc_sbuf_tensor("out_val", [128, 1], f32)

    with tile.TileContext(nc) as tc:
        with (
            tc.tile_pool(name="sb", bufs=1) as sb,
            tc.tile_pool(name="ps_g", bufs=1, space="PSUM") as ps_g,
            tc.tile_pool(name="ps_acc", bufs=1, space="PSUM") as ps_acc,
        ):
            # ---- loads: critical (gtT|M1) on the SP HWDGE ring, the rest
            # on the Pool SWDGE ring so neither waits on the other --------
            pk1 = sb.tile([H, W1], f16)
            nc.sync.dma_start(pk1[:], pack1_d[:])
            pk2 = sb.tile([H, W2], f16)
            nc.gpsimd.dma_start(pk2[:], pack2_d[:])

            # ---- output writeback, prepared early: the SWDGE descriptors
            # are generated during Pool's idle window, so after the final
            # reduce only the trigger (SEQ op + transfer + completion sem)
            # is left -- skipping the ~1.3us HWDGE+DGE serial pipeline.
            # out_val is a RAW sbuf tensor (not a tile): Tile must not see
            # the prep's deferred read, or it inserts a WAR wait on the
            # transfer completion in front of the reduce (deadlock).  The
            # read-after-write ordering is enforced at the trigger instead
            # (signals_writable on the PSUM bank the reduce consumes). ----
            out_ap = out_val.ap()
            wb_idx = sb.tile([128, 1], mybir.dt.int32)
            nc.vector.memset(out_ap, 0.0)
            nc.vector.memset(wb_idx[:], 0)
            nc.gpsimd.kv_writeback(
                out_d[:].rearrange("b (d o n) -> b d o n", o=1, n=1),
                out_ap.rearrange("d (o b n) -> d o b n", b=1, n=1),
                wb_idx[:],
                prepare_only=True,
                sem=dma_sem,
            )

            gtT = pk1[:, 0:2 * H].bitcast(f32)            # [100,100] f32
            m1 = pk1[:, 2 * H:2 * H + RANK * H]           # [100,100R] f16
            ftab = pk2[:, 0:RANK * H]                     # [100,100R] f16
            rm = pk2[:, RANK * H:RANK * H + 2 * H].bitcast(f32)
            prob = pk2[:, RANK * H + 2 * H:RANK * H + 4 * H].bitcast(f32)

            # ---- binarize the transposed mask (stage-1 stationary) ------
            pyt = sb.tile([H, H], f16)
            nc.vector.tensor_scalar(
                pyt[:], gtT, 0.5, None, mybir.AluOpType.is_ge
            )

            # ---- stage 1: G[a, mc] = sum_b pyt[b,a] * m1[b, mc] ---------
            g_ps = ps_g.tile([H, RANK * H], f32)
            nc.tensor.matmul(g_ps[:], pyt[:], m1, start=True, stop=True)

            # G-copy on ACT (otherwise idle; scalar_tensor_tensor is not
            # legal on Pool, so t1n needs DVE and the copy moves off it).
            g16 = sb.tile([H, RANK * H], f16)
            nc.scalar.copy(g16[:], g_ps[:])

            # ---- t1n = px * (-rowsum/N^2) pre-written into the stage-2
            # accumulation bank (matmuls then add on top, start=False) ----
            acc_ps = ps_acc.tile([H, H], f32)
            nc.vector.scalar_tensor_tensor(
                acc_ps[:],
                prob,
                0.5,
                rm,
                op0=mybir.AluOpType.is_ge,
                op1=mybir.AluOpType.mult,
            )

            # ---- stage 2: acc += sum_m ftab_m^T @ G_m -------------------
            for k in range(RANK):
                nc.tensor.matmul(
                    acc_ps[:],
                    ftab[:, k * H:(k + 1) * H],
                    g16[:, k * H:(k + 1) * H],
                    start=False,
                    stop=(k == RANK - 1),
                    skip_group_check=True,
                )

            # ---- final: sum_i |acc_i| (scales already folded).  Pool's
            # partition-axis reduce cannot read PSUM, so: DVE abs-reduce
            # along the free axis (PSUM -> [100,1] SBUF), then a tiny Pool
            # partition reduce to the scalar -- no ones-matmul. -----------
            rowsums = sb.tile([H, 1], f32)
            nc.vector.tensor_reduce(
                rowsums[:],
                acc_ps[:],
                axis=mybir.AxisListType.X,
                op=mybir.AluOpType.add,
                apply_absolute_value=True,
            )
            nc.gpsimd.tensor_reduce(
                out_ap[0:1, 0:1],
                rowsums[:],
                axis=mybir.AxisListType.XYZWC,
                op=mybir.AluOpType.add,
            )
            # Fire the pre-built writeback.  signals_writable=[rowsums]
            # gives the trigger a WAR dep on the Pool reduce's read so it
            # cannot fire before the scalar is written.  The epilogue's
            # SWDGE-queue drain (DMASW lane sem, rewritten below) holds
            # program end until the transfer completes.
            nc.gpsimd.trigger_dma(signals_writable=[rowsums[:]])

    nc.compile()

    # Post-compile surgery (compile regenerates event-semaphore waits, so
    # this must run on the final BIR).
    #
    # Tile schedules the kv-writeback prep on a dedicated DMASW lane and
    # makes the epilogue wait for that lane's completion sem (+16), but the
    # descriptor-baked completion sem is the one passed as `sem=` above.
    # Rewrite the prep's completion update to target the (otherwise
    # orphaned) DMASW lane sem so the epilogue wait is satisfied by the
    # actual transfer completion.
    updated_ids = set()
    waited = {}
    prep = None
    for bb in nc.m.functions[0].blocks:
        for inst in bb.instructions:
            if type(inst).__name__ == "InstKVWritebackAnt":
                prep = inst
            si = inst.sync_info
            if si is None:
                continue
            for u in si.on_update:
                updated_ids.add(u.id)
            for w in si.on_wait:
                if w.ant_name and w.ant_name.startswith("DMASW"):
                    waited[w.id] = w.ant_name
    orphans = {i: n for i, n in waited.items() if i not in updated_ids}
    assert prep is not None and len(orphans) == 1, (prep, orphans)
    ((oid, oname),) = orphans.items()
    si = prep.sync_info
    old = list(si.on_update)
    si.on_update = [
        mybir.SyncUpdate(
            sync_type="semaphore",
            id=oid,
            ant_name=oname,
            update_mode="sem-add-imm",
            update_value=16,
            update_reg=None,
        )
    ] + old[1:]

    # Tile also guards the Pool Q7 library reload (for the XYZWC reduce)
    # behind "SWDGE queue drained" -- circular with a prepared-untriggered
    # DMA.  The guard is unnecessary for this pattern (the descriptors
    # live in the SBUF ring, not Q7 memory): strip the orphan-lane wait
    # from body-block event semaphores.  The epilogue's drain waits (which
    # hold program end until the transfer lands) are left untouched.
    for bb in nc.m.functions[0].blocks:
        if not (bb.name.startswith("tile_context") and not bb.name.endswith("_end")):
            continue
        for inst in bb.instructions:
            if type(inst).__name__ != "InstEventSemaphore":
                continue
            si2 = inst.sync_info
            if si2 is None:
                continue
            kept = [w for w in si2.on_wait if w.id != oid]
            if len(kept) != len(list(si2.on_wait)):
                si2.on_wait = kept

    # The writeback-completion wait (orphan lane >= 16) sits on an SP
    # event-semaphore ahead of SP's barrier join, serializing the 900ns
    # completion latency with the whole barrier round.  Pool's gather wait
    # is the true last step of the round, so observe the completion there
    # instead: move the wait from the SP event-sem onto Pool's gather
    # event-sem (2 wait conditions per event-sem is within HW limits).
    for bb in nc.m.functions[0].blocks:
        if not bb.name.endswith("_end"):
            continue
        sp_waiter = None
        pool_gather = None
        for inst in bb.instructions:
            if type(inst).__name__ != "InstEventSemaphore":
                continue
            si3 = inst.sync_info
            if si3 is None:
                continue
            waits = list(si3.on_wait)
            if sp_waiter is None and any(w.id == oid for w in waits):
                sp_waiter = inst
            if (
                pool_gather is None
                and any("gather" in (w.ant_name or "") for w in waits)
            ):
                pool_gather = inst
        assert sp_waiter is not None and pool_gather is not None
        si_sp = sp_waiter.sync_info
        si_sp.on_wait = [w for w in si_sp.on_wait if w.id != oid]
        # The SP wait on the trigger's sequencer tick (Pool_sequencer_*)
        # fires 900ns after the transfer too (trigger updates ride the DMA
        # overhead path) and is subsumed by the gather's DMASW wait, which
        # is strictly later.  Strip it so SP joins the barrier early.
        for inst in bb.instructions:
            if type(inst).__name__ != "InstEventSemaphore":
                continue
            si4 = inst.sync_info
            if si4 is None:
                continue
            kept4 = [
                w for w in si4.on_wait
                if not (w.ant_name or "").startswith("Pool_sequencer")
            ]
            if len(kept4) != len(list(si4.on_wait)):
                si4.on_wait = kept4
        si_pg = pool_gather.sync_info
        si_pg.on_wait = list(si_pg.on_wait) + [
            mybir.SyncWait(
                sync_type="semaphore",
                id=oid,
                ant_name=oname,
                wait_mode="sem-ge-imm",
                wait_value=16,
                wait_reg=None,
            )
        ]

    # The epilogue runs TWO all-engine barrier rounds: one before the Pool
    # sem-cleanup ISA (needed -- it guarantees quiescence), and a second
    # after it whose only purpose is to delay every engine's halt until
    # the cleanup ran.  Engines halting at different times is fine (the
    # program is over; cleanup still completes before Pool halts), so
    # neuter the second round's synchronization.
    for bb in nc.m.functions[0].blocks:
        if not bb.name.endswith("_end"):
            continue
        seen_isa = False
        for inst in bb.instructions:
            tn = type(inst).__name__
            if tn == "InstISA":
                seen_isa = True
                continue
            if seen_isa and tn in ("InstDrain", "InstEventSemaphore"):
                si3 = inst.sync_info
                if si3 is not None:
                    si3.on_wait = []
                    si3.on_update = []

    return nc


_STATE = {}


def _get_state():
    if not _STATE:
        _STATE["consts"] = _host_constants()
        _STATE["nc"] = _build_module()
    return _STATE


def _in_maps(prob_map, gt_map):
    st = _get_state()
    m1, const2 = st["consts"]
    prob = np.asarray(prob_map, dtype=np.float32).reshape(H, H)
    gt = np.asarray(gt_map, dtype=np.float32).reshape(H, H)
    gtT = np.ascontiguousarray(gt.T)

    pack1 = np.concatenate([gtT.view(np.float16), m1], axis=1)
    pack2 = const2.copy()
    pack2[:, RANK * H + 2 * H:RANK * H + 4 * H] = prob.view(np.float16)
    return [{"pack1": np.ascontiguousarray(pack1),
             "pack2": np.ascontiguousarray(pack2)}]


def _run(prob_map, gt_map, trace=False, **spmd_kwargs):
    from concourse import bass_utils

    st = _get_state()
    in_maps = _in_maps(prob_map, gt_map)
    res = bass_utils.run_bass_kernel_spmd(
        st["nc"], in_maps, core_ids=[0], trace=trace, **spmd_kwargs,
    )
    value = np.float32(np.asarray(res.results[0]["out"]).ravel()[0])
    return value, res


def kernel(prob_map, gt_map):
    value, _ = _run(prob_map, gt_map, trace=False)
    return np.asarray(value, dtype=np.float32)


# revision 38
# speedup vs baseline: 1.0527x; 1.0115x over previous
"""Trainium2 Bass kernel for nn_HausdorffDistance_28406913696124.

Math (reference):
    px = (prob_map[0].ravel() >= 0.5)                 # [N], N = 100*100
    py = (gt_map.ravel()   >= 0.5)                    # [N]
    D[i,j] = euclid dist between grid points i, j     # [N, N] constant!
    loss   = mean_i | px_i * mean_j D[i,j] - (D @ py)_i / N |

Key structure:
  * rowmean_i = mean_j D[i,j] is a pure geometric constant -> host table.
  * (D @ py) is a 2D correlation of the binary mask PY with the 199x199
    kernel K(u,v) = sqrt(u^2 + v^2).  K is numerically low-rank: with a
    displacement-count weighted SVD, rank 2 already reproduces the final
    scalar to ~1e-5 relative (tolerance is 2e-2).  K ~= sum_m f_m(u) g_m(v)
    turns the 2D correlation into two passes of 1D Toeplitz matmuls:

        G_m = PY^T-contraction:  G_m[a, c] = sum_b PYT[b, a] * g_m(c - b)
        T2[r, c] = sum_m sum_a f_m(r - a) * G_m[a, c]

    Stage 1 is ONE matmul (stationary = binarized transposed mask, moving =
    the [100, 100*M] constant Toeplitz table); stage 2 is M accumulating
    matmuls with constant stationaries.  The gt transpose is done on host so
    no on-device transposes are needed.

  * All scalar factors (the two 1/N means) are folded into the constant
    tables (1e-4 into each of the stage-1/stage-2 tables, -1e-8 into the
    row-sum table), so after |.| row-reduction a single ones-matmul across
    partitions yields the final loss directly -- no post-scale.

  * term1 is folded in by pre-writing t1n = px * (-1e-8 * rowsum) into the
    stage-2 PSUM accumulation bank before the matmuls (start=False), so the
    bank ends up holding diff = 1e-8*(T2sum - px*rowsumN) elementwise.

Single core, no collectives: the whole problem is ~0.5 MFLOP, so the 8-core
AllReduce floor (~5us) dwarfs any compute sharding gain.  The kernel is
latency-bound: two parallel input DMAs (SP ring + Pool SWDGE ring), ~0.6us
of matmul/DVE work, one 4-byte output DMA.
"""

import sys

import numpy as np

sys.path.insert(0, "/opt/trn_rl_repo")

H = 100
N = H * H
RANK = 1      # separable ranks of the distance kernel (1 -> ~4e-4 rel err,
              # 2 -> ~2e-5; tolerance is 2e-2)
S_A = 1e-4    # scale folded into the stage-1 moving table
S_B = 1e-4    # scale folded into the stage-2 stationary table
PAD2 = 8      # f16 cols holding the f32 ones column (4) + pad


def _host_constants():
    """Geometry-only constant tables (input independent)."""
    idx = np.arange(H)

    # Displacement-count weighted SVD of K(u,v) = sqrt(u^2+v^2) on
    # [-99,99]^2: weight sqrt(100-|u|) per axis makes the truncation error
    # small exactly where displacements are frequent.
    u = np.arange(-(H - 1), H)
    K = np.sqrt((u[:, None] ** 2 + u[None, :] ** 2).astype(np.float64))
    wu = np.sqrt((H - np.abs(u)).astype(np.float64))
    Uw, S, Vtw = np.linalg.svd(wu[:, None] * K * wu[None, :])
    ffac = (Uw * np.sqrt(S)[None, :]) / wu[:, None]    # [199, m]
    gfac = (Vtw.T * np.sqrt(S)[None, :]) / wu[:, None]  # [199, m]

    # Toeplitz tables: offs[a, r] = (r - a) + 99
    offs = (idx[None, :] - idx[:, None]) + (H - 1)
    m1 = np.concatenate(
        [gfac[:, k][offs] * S_A for k in range(RANK)], axis=1
    ).astype(np.float16)                               # [100, 100*RANK]
    ftab = np.concatenate(
        [ffac[:, k][offs] * S_B for k in range(RANK)], axis=1
    ).astype(np.float16)                               # [100, 100*RANK]

    # rowsum[r,c] = sum_j D[i,j] (i = r*100+c), f64-exact, scaled by -1e-8
    # so the DVE write into the accumulation bank is already -px*rowsum/N^2.
    absdiff = np.abs(idx[:, None] - idx[None, :])
    q = np.sqrt((idx[:, None] ** 2 + idx[None, :] ** 2).astype(np.float64))
    cnt = np.zeros((H, H))
    np.add.at(cnt, (idx[:, None], absdiff), 1.0)
    rm_tbl = (-(cnt @ q @ cnt.T) * (S_A * S_B)).astype(np.float32)

    # pack2 = [ftab | rm(f32) | prob placeholder(f32)]
    const2 = np.concatenate(
        [
            ftab,
            rm_tbl.view(np.float16),
            np.zeros((H, 2 * H), dtype=np.float16),  # prob slot
        ],
        axis=1,
    )
    return m1, const2


def _build_module():
    import concourse.bacc as bacc
    import concourse.bass as bass_mod
    import concourse.mybir as mybir
    import concourse.tile as tile

    f32 = mybir.dt.float32
    f16 = mybir.dt.float16

    # Bass.__init__ emits four const-AP memsets, all on Pool; Pool then
    # joins the prologue all-engine barrier ~400ns after everyone else,
    # delaying the whole program.  Spread the two f32 consts across DVE
    # and Pool, and skip the bf16-1.0 / uint8-127 fills entirely -- no
    # instruction in this program touches those dtypes, so the (still
    # allocated) regions are never read.
    _orig_memset = bass_mod.BassGpSimd.memset
    _ctr = [0]

    def _memset_rr(self, ap, constant):
        _ctr[0] += 1
        if _ctr[0] == 4:
            return None  # uint8-127 (mx-quant identity scale): unused
        if _ctr[0] % 2:
            return bass_mod.BassVectorEngine.memset(self.bass.vector, ap, constant)
        return _orig_memset(self, ap, constant)

    bass_mod.BassGpSimd.memset = _memset_rr
    try:
        nc = bacc.Bacc(
            "TRN2",
            target_bir_lowering=False,
            debug=False,
            enable_asserts=False,
            num_devices=1,
        )
    finally:
        bass_mod.BassGpSimd.memset = _orig_memset

    W1 = 2 * H + RANK * H          # pack1: gtT (f32 as 2H f16) | M1
    W2 = RANK * H + 4 * H          # pack2: ftab | rm | prob
    pack1_d = nc.dram_tensor("pack1", [H, W1], f16, kind="ExternalInput")
    pack2_d = nc.dram_tensor("pack2", [H, W2], f16, kind="ExternalInput")
    # kv_writeback stores a full 128-partition column; slot 0 is the answer.
    out_d = nc.dram_tensor("out", [1, 128], f32, kind="ExternalOutput")

    dma_sem = nc.alloc_semaphore("out_wb_done")
    out_val = nc.alloc_sbuf_tensor("out_val", [128, 1], f32)

    with tile.TileContext(nc) as tc:
        with (
            tc.tile_pool(name="sb", bufs=1) as sb,
            tc.tile_pool(name="ps_g", bufs=1, space="PSUM") as ps_g,
            tc.tile_pool(name="ps_acc", bufs=1, space="PSUM") as ps_acc,
        ):
            # ---- loads: critical (gtT|M1) on the SP HWDGE ring, the rest
            # on the Pool SWDGE ring so neither waits on the other --------
            pk1 = sb.tile([H, W1], f16)
            nc.sync.dma_start(pk1[:], pack1_d[:])
            pk2 = sb.tile([H, W2], f16)
            nc.gpsimd.dma_start(pk2[:], pack2_d[:])

            # ---- output writeback, prepared early: the SWDGE descriptors
            # are generated during Pool's idle window, so after the final
            # reduce only the trigger (SEQ op + transfer + completion sem)
            # is left -- skipping the ~1.3us HWDGE+DGE serial pipeline.
            # out_val is a RAW sbuf tensor (not a tile): Tile must not see
            # the prep's deferred read, or it inserts a WAR wait on the
            # transfer completion in front of the reduce (deadlock).  The
            # read-after-write ordering is enforced at the trigger instead
            # (signals_writable on the PSUM bank the reduce consumes). ----
            out_ap = out_val.ap()
            wb_idx = sb.tile([128, 1], mybir.dt.int32)
            nc.vector.memset(out_ap, 0.0)
            nc.vector.memset(wb_idx[:], 0)
            nc.gpsimd.kv_writeback(
                out_d[:].rearrange("b (d o n) -> b d o n", o=1, n=1),
                out_ap.rearrange("d (o b n) -> d o b n", b=1, n=1),
                wb_idx[:],
                prepare_only=True,
                sem=dma_sem,
            )

            gtT = pk1[:, 0:2 * H].bitcast(f32)            # [100,100] f32
            m1 = pk1[:, 2 * H:2 * H + RANK * H]           # [100,100R] f16
            ftab = pk2[:, 0:RANK * H]                     # [100,100R] f16
            rm = pk2[:, RANK * H:RANK * H + 2 * H].bitcast(f32)
            prob = pk2[:, RANK * H + 2 * H:RANK * H + 4 * H].bitcast(f32)

            # ---- binarize the transposed mask (stage-1 stationary) ------
            pyt = sb.tile([H, H], f16)
            nc.vector.tensor_scalar(
                pyt[:], gtT, 0.5, None, mybir.AluOpType.is_ge
            )

            # ---- stage 1: G[a, mc] = sum_b pyt[b,a] * m1[b, mc] ---------
            g_ps = ps_g.tile([H, RANK * H], f32)
            nc.tensor.matmul(g_ps[:], pyt[:], m1, start=True, stop=True)

            # G-copy on ACT (otherwise idle; scalar_tensor_tensor is not
            # legal on Pool, so t1n needs DVE and the copy moves off it).
            g16 = sb.tile([H, RANK * H], f16)
            nc.scalar.copy(g16[:], g_ps[:])

            # ---- t1n = px * (-rowsum/N^2) pre-written into the stage-2
            # accumulation bank (matmuls then add on top, start=False) ----
            acc_ps = ps_acc.tile([H, H], f32)
            nc.vector.scalar_tensor_tensor(
                acc_ps[:],
                prob,
                0.5,
                rm,
                op0=mybir.AluOpType.is_ge,
                op1=mybir.AluOpType.mult,
            )

            # ---- stage 2: acc += sum_m ftab_m^T @ G_m -------------------
            for k in range(RANK):
                nc.tensor.matmul(
                    acc_ps[:],
                    ftab[:, k * H:(k + 1) * H],
                    g16[:, k * H:(k + 1) * H],
                    start=False,
                    stop=(k == RANK - 1),
                    skip_group_check=True,
                )

            # ---- final: sum_i |acc_i| (scales already folded).  Pool's
            # partition-axis reduce cannot read PSUM, so: DVE abs-reduce
            # along the free axis (PSUM -> [100,1] SBUF), then a tiny Pool
            # partition reduce to the scalar -- no ones-matmul. -----------
            rowsums = sb.tile([H, 1], f32)
            nc.vector.tensor_reduce(
                rowsums[:],
                acc_ps[:],
                axis=mybir.AxisListType.X,
                op=mybir.AluOpType.add,
                apply_absolute_value=True,
            )
            nc.gpsimd.tensor_reduce(
                out_ap[0:1, 0:1],
                rowsums[:],
                axis=mybir.AxisListType.XYZWC,
                op=mybir.AluOpType.add,
            )
            # Fire the pre-built writeback.  signals_writable=[rowsums]
            # gives the trigger a WAR dep on the Pool reduce's read so it
            # cannot fire before the scalar is written.  The epilogue's
            # SWDGE-queue drain (DMASW lane sem, rewritten below) holds
            # program end until the transfer completes.
            nc.gpsimd.trigger_dma(signals_writable=[rowsums[:]])

    nc.compile()

    # Post-compile surgery (compile regenerates event-semaphore waits, so
    # this must run on the final BIR).
    #
    # Tile schedules the kv-writeback prep on a dedicated DMASW lane and
    # makes the epilogue wait for that lane's completion sem (+16), but the
    # descriptor-baked completion sem is the one passed as `sem=` above.
    # Rewrite the prep's completion update to target the (otherwise
    # orphaned) DMASW lane sem so the epilogue wait is satisfied by the
    # actual transfer completion.
    updated_ids = set()
    waited = {}
    prep = None
    for bb in nc.m.functions[0].blocks:
        for inst in bb.instructions:
            if type(inst).__name__ == "InstKVWritebackAnt":
                prep = inst
            si = inst.sync_info
            if si is None:
                continue
            for u in si.on_update:
                updated_ids.add(u.id)
            for w in si.on_wait:
                if w.ant_name and w.ant_name.startswith("DMASW"):
                    waited[w.id] = w.ant_name
    orphans = {i: n for i, n in waited.items() if i not in updated_ids}
    assert prep is not None and len(orphans) == 1, (prep, orphans)
    ((oid, oname),) = orphans.items()
    si = prep.sync_info
    old = list(si.on_update)
    si.on_update = [
        mybir.SyncUpdate(
            sync_type="semaphore",
            id=oid,
            ant_name=oname,
            update_mode="sem-add-imm",
            update_value=16,
            update_reg=None,
        )
    ] + old[1:]

    # Tile also guards the Pool Q7 library reload (for the XYZWC reduce)
    # behind "SWDGE queue drained" -- circular with a prepared-untriggered
    # DMA.  The guard is unnecessary for this pattern (the descriptors
    # live in the SBUF ring, not Q7 memory): strip the orphan-lane wait
    # from body-block event semaphores.  The epilogue's drain waits (which
    # hold program end until the transfer lands) are left untouched.
    for bb in nc.m.functions[0].blocks:
        if not (bb.name.startswith("tile_context") and not bb.name.endswith("_end")):
            continue
        for inst in bb.instructions:
            if type(inst).__name__ != "InstEventSemaphore":
                continue
            si2 = inst.sync_info
            if si2 is None:
                continue
            kept = [w for w in si2.on_wait if w.id != oid]
            if len(kept) != len(list(si2.on_wait)):
                si2.on_wait = kept

    # The writeback-completion wait (orphan lane >= 16) sits on an SP
    # event-semaphore ahead of SP's barrier join, serializing the 900ns
    # completion latency with the whole barrier round.  Pool's gather wait
    # is the true last step of the round, so observe the completion there
    # instead: move the wait from the SP event-sem onto Pool's gather
    # event-sem (2 wait conditions per event-sem is within HW limits).
    for bb in nc.m.functions[0].blocks:
        if not bb.name.endswith("_end"):
            continue
        sp_waiter = None
        pool_gather = None
        for inst in bb.instructions:
            if type(inst).__name__ != "InstEventSemaphore":
                continue
            si3 = inst.sync_info
            if si3 is None:
                continue
            waits = list(si3.on_wait)
            if sp_waiter is None and any(w.id == oid for w in waits):
                sp_waiter = inst
            if (
                pool_gather is None
                and any("gather" in (w.ant_name or "") for w in waits)
            ):
                pool_gather = inst
        assert sp_waiter is not None and pool_gather is not None
        si_sp = sp_waiter.sync_info
        si_sp.on_wait = [w for w in si_sp.on_wait if w.id != oid]
        # The SP wait on the trigger's sequencer tick (Pool_sequencer_*)
        # fires 900ns after the transfer too (trigger updates ride the DMA
        # overhead path) and is subsumed by the gather's DMASW wait, which
        # is strictly later.  Strip it so SP joins the barrier early.
        for inst in bb.instructions:
            if type(inst).__name__ != "InstEventSemaphore":
                continue
            si4 = inst.sync_info
            if si4 is None:
                continue
            kept4 = [
                w for w in si4.on_wait
                if not (w.ant_name or "").startswith("Pool_sequencer")
            ]
            if len(kept4) != len(list(si4.on_wait)):
                si4.on_wait = kept4
        si_pg = pool_gather.sync_info
        si_pg.on_wait = list(si_pg.on_wait) + [
            mybir.SyncWait(
                sync_type="semaphore",
                id=oid,
                ant_name=oname,
                wait_mode="sem-ge-imm",
                wait_value=16,
                wait_reg=None,
            )
        ]

    # The epilogue runs TWO all-engine barrier rounds: one before the Pool
    # sem-cleanup ISA (needed -- it guarantees quiescence), and a second
    # after it whose only purpose is to delay every engine's halt until
    # the cleanup ran.  Engines halting at different times is fine (the
    # program is over; cleanup still completes before Pool halts), so
    # neuter the second round's synchronization.
    for bb in nc.m.functions[0].blocks:
        if not bb.name.endswith("_end"):
            continue
        seen_isa = False
        for inst in bb.instructions:
            tn = type(inst).__name__
            if tn == "InstISA":
                seen_isa = True
                continue
            if seen_isa and tn in ("InstDrain", "InstEventSemaphore"):
                si3 = inst.sync_info
                if si3 is not None:
                    si3.on_wait = []
                    si3.on_update = []

    return nc


_STATE = {}


def _get_state():
    if not _STATE:
        _STATE["consts"] = _host_constants()
        _STATE["nc"] = _build_module()
    return _STATE


def _in_maps(prob_map, gt_map):
    st = _get_state()
    m1, const2 = st["consts"]
    prob = np.asarray(prob_map, dtype=np.float32).reshape(H, H)
    gt = np.asarray(gt_map, dtype=np.float32).reshape(H, H)
    gtT = np.ascontiguousarray(gt.T)

    pack1 = np.concatenate([gtT.view(np.float16), m1], axis=1)
    pack2 = const2.copy()
    pack2[:, RANK * H + 2 * H:RANK * H + 4 * H] = prob.view(np.float16)
    return [{"pack1": np.ascontiguousarray(pack1),
             "pack2": np.ascontiguousarray(pack2)}]


def _run(prob_map, gt_map, trace=False, **spmd_kwargs):
    from concourse import bass_utils

    st = _get_state()
    in_maps = _in_maps(prob_map, gt_map)
    res = bass_utils.run_bass_kernel_spmd(
        st["nc"], in_maps, core_ids=[0], trace=trace, **spmd_kwargs,
    )
    value = np.float32(np.asarray(res.results[0]["out"]).ravel()[0])
    return value, res


def kernel(prob_map, gt_map):
    value, _ = _run(prob_map, gt_map, trace=False)
    return np.asarray(value, dtype=np.float32)
